# revision 29
# baseline (speedup 1.0000x reference)
"""Adversarial-MMD loss (nn_Advmmd) on 8 Trainium2 NeuronCores via Bass/Tile.

Math (eval mode, lamb=0):
  adv:  the discriminator is Linear(2048,128) -> Dropout(eval) -> Linear(128,1)
        with NO nonlinearity, so it collapses to a single linear functional
        z = x.w + beta with w = W2@W1 [2048], beta = W2@b1 + b2.
        adv_loss = 0.5*(mean log(1+exp(-z_src)) + mean log(1+exp(+z_tgt)))
  mmd:  total = [source;target] [8192,2048]; L2_ij = sq_i + sq_j - 2 G_ij with
        G = total@total.T;  bandwidth bw = sum(L2)/(n^2-n)/4 where
        sum(L2) = 2n*sum(sq) - 2*||sum_j total_j||^2 (exact identity);
        K = sum_{p=0..4} exp(-L2/(bw*2^p));
        loss = mean K[XX] + mean K[YY] - mean K[XY] - mean K[YX].

Distribution: data-parallel over Gram rows.  Core c owns 1024 rows; it
computes its [1024, 8192] Gram block in bf16 on the PE (fp32 accumulate),
applies the five Gaussian kernels on the Scalar engine (exp with
per-partition scale/bias; row sums come free via accum_out) and reduces to
two scalars (left/right half block sums).  Row norms and column sums are
exchanged with a single small AllGather; everything else is local.

Row-tile loads for the stats phase use a strided partition map
(partition p <-> local row p*8+mt) so the per-core stats land contiguously
in DRAM for the collective; all sums are order invariant.

The pipeline drains PE's PSUM through a plain copy into SBUF tiles, so the
matmul stream never waits for the collective; the -sq_j/2 column correction
is added in-place afterwards, once the AllGather lands.
"""

import numpy as np
import ml_dtypes

N_CORES = 8
B = 4096
D = 2048
NT = 2 * B            # 8192 total rows
RPC = NT // N_CORES   # 1024 rows per core
MT = RPC // 128       # 8 m-tiles per core
KT = D // 128         # 16 k-tiles
NCHUNK = 512
NQUAD = NT // (4 * NCHUNK)   # 4 groups of four 512-column chunks
KP = KT // 2                 # 8 double-row k-pairs
KERNEL_NUM = 5
CCW = RPC + D         # per-core AllGather payload: [sq (1024) | s (2048)]

_cached = {}


def _build_module():
    import concourse.bass as bass
    import concourse.tile as tile
    import concourse.mybir as mybir
    from concourse import bacc

    f32 = mybir.dt.float32
    bf16 = mybir.dt.bfloat16
    AF = mybir.ActivationFunctionType
    ALU = mybir.AluOpType

    nc = bacc.Bacc(
        "TRN2",
        target_bir_lowering=False,
        debug=False,
        enable_asserts=False,
        num_devices=N_CORES,
    )

    fp8 = mybir.dt.float8e4
    totalT_d = nc.dram_tensor("totalT", [D, NT], fp8, kind="ExternalInput")
    blockT_d = nc.dram_tensor("blockT", [D, RPC], fp8, kind="ExternalInput")
    rows_d = nc.dram_tensor("rows", [RPC, D], fp8, kind="ExternalInput")
    rowsbf_d = nc.dram_tensor("rowsbf", [RPC, D], bf16, kind="ExternalInput")
    W1_d = nc.dram_tensor("w1", [128, D], f32, kind="ExternalInput")
    W2T_d = nc.dram_tensor("w2t", [128, 1], f32, kind="ExternalInput")
    b1_d = nc.dram_tensor("b1c", [128, 1], f32, kind="ExternalInput")
    b2_d = nc.dram_tensor("b2c", [1, 1], f32, kind="ExternalInput")
    sgn_d = nc.dram_tensor("sgn", [128, 1], f32, kind="ExternalInput")
    lad_d = nc.dram_tensor("ladder", [1, KERNEL_NUM], f32, kind="ExternalInput")
    out_d = nc.dram_tensor("out", [1, 8], f32, kind="ExternalOutput")

    rg = [list(range(N_CORES))]

    with tile.TileContext(nc) as tc:
        with (
            tc.tile_pool(name="big", bufs=1) as big,
            tc.tile_pool(name="rhsp", bufs=7) as rhsp,
            tc.tile_pool(name="x2p", bufs=6) as x2p,
            tc.tile_pool(name="ejp", bufs=1) as ejp,
            tc.tile_pool(name="e1p", bufs=2) as e1p,
            tc.tile_pool(name="e0p", bufs=1) as e0p,
            tc.tile_pool(name="smalls", bufs=1) as smalls,
            tc.tile_pool(name="gpsum", bufs=3, space="PSUM") as gpsum,
            tc.tile_pool(name="spsum", bufs=1, space="PSUM") as spsum,
            tc.tile_pool(name="dram", bufs=1, space="DRAM") as dram,
            tc.tile_pool(name="prol", bufs=1) as prol,
            tc.tile_pool(name="rowp", bufs=2) as rowp,
        ):
            # ---------- persistent tiles ----------
            lhsT_all = big.tile([128, KP, 2, RPC], fp8)     # 16KB/p
            colsq = big.tile([128, NT], f32)                # 32KB/p
            accs = big.tile([128, NQUAD * MT * KERNEL_NUM], f32)
            rowsq = smalls.tile([128, MT], f32)
            bias_all = smalls.tile([128, KERNEL_NUM * MT], f32)
            scales5 = smalls.tile([128, KERNEL_NUM], f32)
            vec5b = smalls.tile([128, KERNEL_NUM], f32)
            zcols = smalls.tile([128, MT], f32)
            zraw = smalls.tile([128, MT], f32)
            ecols = smalls.tile([128, MT], f32)
            lncols = smalls.tile([128, MT], f32)
            fin = smalls.tile([128, 4], f32)
            ones_8 = smalls.tile([128, 1], fp8)
            ones_f = smalls.tile([128, 1], f32)
            sgn_sb = smalls.tile([128, 1], f32)
            lad_sb = smalls.tile([1, KERNEL_NUM], f32)
            vec5 = smalls.tile([1, KERNEL_NUM], f32)
            sqsum = smalls.tile([1, 1], f32)
            s2s = smalls.tile([1, 1], f32)
            t1s = smalls.tile([1, 1], f32)
            bws = smalls.tile([1, 1], f32)
            bwinv = smalls.tile([1, 1], f32)
            beta_sb = smalls.tile([1, 1], f32)
            betab = smalls.tile([128, 1], f32)
            outsb = smalls.tile([4, 1], f32)

            # ---------- prologue tiles ----------
            W1_sb = prol.tile([128, D], f32)                # 8KB/p
            W2T_sb = prol.tile([128, 1], f32)
            b1_sb = prol.tile([128, 1], f32)
            b2_sb = prol.tile([1, 1], f32)
            wb = prol.tile([128, D], bf16)                  # 4KB/p
            w_sb = prol.tile([1, D], bf16)
            s2d_sb = prol.tile([128, KT], f32)
            bias_src = prol.tile([128, MT], f32)
            s3v = prol.tile([128, N_CORES, KT], f32)
            s_glob = prol.tile([128, KT], f32)
            sprod = prol.tile([128, KT], f32)
            sprod_r = prol.tile([128, 1], f32)
            junk2 = prol.tile([128, D], bf16)               # 4KB/p
            junk3 = prol.tile([128, D], bf16)               # 4KB/p

            # ---------- DRAM collective buffers ----------
            cc_in = dram.tile([CCW], f32)
            cc_out = dram.tile([N_CORES * CCW], f32, addr_space="Shared")

            # ---------- constant + data loads ----------
            nc.gpsimd.dma_start(W1_sb[:], W1_d[:, :])
            nc.gpsimd.dma_start(W2T_sb[:], W2T_d[:, :])
            nc.gpsimd.dma_start(b1_sb[:], b1_d[:, :])
            nc.gpsimd.dma_start(b2_sb[:], b2_d[:, :])
            nc.gpsimd.dma_start(sgn_sb[:], sgn_d[:, :])
            nc.gpsimd.dma_start(lad_sb[:], lad_d[:, :])
            nc.vector.memset(ones_8[:], 1.0)
            nc.vector.memset(ones_f[:], 1.0)
            nc.vector.memset(fin[:, 3:4], 0.0)
            nc.sync.dma_start(
                lhsT_all[:],
                blockT_d.ap().rearrange("(kp two p) m -> p kp two m", p=128, two=2),
            )

            # ---------- local stats: sq_i (ACT Square) and s (PE) ----------
            # rows are host-permuted so local row index r = p*MT + m lives at
            # partition p, slot m -> the stats DMA out is contiguous
            s_ps = spsum.tile([128, KT], f32, tag="sps")
            rows_r = rows_d.ap().rearrange("(p m) k -> m p k", m=MT)
            rowsbf_r = rowsbf_d.ap().rearrange("(p m) k -> m p k", m=MT)
            for mt in range(MT):
                rt = rowp.tile([128, D], fp8, tag="rt8")
                nc.sync.dma_start(rt[:], rows_r[mt])
                nc.scalar.activation(
                    junk2[:], rt[:], AF.Square, bias=0.0, scale=1.0,
                    accum_out=rowsq[:, mt : mt + 1],
                )
                for kt in range(KT):
                    nc.tensor.matmul(
                        s_ps[:, kt : kt + 1],
                        rt[:, bass.ts(kt, 128)],
                        ones_8[:],
                        start=(mt == 0),
                        stop=(mt == MT - 1),
                    )
            nc.scalar.copy(s2d_sb[:], s_ps[:])

            # ---------- one AllGather: [sq_perm (1024) | s (2048)] ----------
            # both input DMAs are contiguous 32/64B lines per partition
            rowsqh = prol.tile([128, MT], f32)
            nc.vector.tensor_scalar_mul(rowsqh[:], rowsq[:], -0.5)
            nc.gpsimd.dma_start(
                cc_in[0:RPC].rearrange("(p m) -> p m", p=128), rowsqh[:]
            )
            nc.gpsimd.dma_start(
                cc_in[RPC:CCW].rearrange("(p kt) -> p kt", p=128), s2d_sb[:]
            )
            nc.gpsimd.collective_compute(
                "AllGather",
                ALU.bypass,
                replica_groups=rg,
                ins=[cc_in.opt()],
                outs=[cc_out.opt()],
            )
            cc_view = cc_out.rearrange("(c w) -> c w", c=N_CORES)
            # colsq[p, j] = sq_j  (j = c*RPC + r, r contiguous inside block)
            nc.scalar.dma_start(
                colsq[:].rearrange("p (c r) -> p c r", c=N_CORES),
                cc_view[None, :, 0:RPC].broadcast_to((128, N_CORES, RPC)),
            )
            # s parts land as [p, c, kt] (contiguous 64B lines per c)
            nc.scalar.dma_start(
                s3v[:],
                cc_view[:, RPC:CCW].rearrange("c (p kt) -> p c kt", p=128),
            )
            # per-(m-tile) bias layout of our own sq, from the local cc input:
            # bias_src[i, m] = sq(row m*128+i)
            nc.scalar.dma_start(
                bias_src[:],
                cc_in[0:RPC].rearrange("(m i) -> i m", i=128),
            )

            # sq (already scaled by -1/2) in a multi-partition layout for
            # the global sum: [128, c, m] lines of 32B
            sqt = prol.tile([128, N_CORES, MT], f32)
            nc.scalar.dma_start(
                sqt[:],
                cc_view[:, 0:RPC].rearrange("c (p m) -> p c m", p=128),
            )

            # ---------- bandwidth ----------
            nc.vector.tensor_reduce(
                s_glob[:], s3v[:].rearrange("p c kt -> p kt c"),
                axis=mybir.AxisListType.X, op=ALU.add,
            )
            sqr = prol.tile([128, 1], f32)
            nc.vector.tensor_reduce(
                sqr[:], sqt[:], axis=mybir.AxisListType.XY, op=ALU.add
            )
            t2_ps = spsum.tile([1, 1], f32, tag="sps")
            nc.tensor.matmul(t2_ps[:], sqr[:], ones_f[:], start=True, stop=True)
            nc.vector.tensor_scalar_mul(sqsum[:], t2_ps[:], -2.0)
            nc.vector.tensor_tensor(
                out=sprod[:], in0=s_glob[:], in1=s_glob[:], op=ALU.mult
            )
            nc.vector.tensor_reduce(
                sprod_r[:], sprod[:], axis=mybir.AxisListType.X, op=ALU.add
            )
            s2_ps = spsum.tile([1, 1], f32, tag="sps")
            nc.tensor.matmul(s2_ps[:], sprod_r[:], ones_f[:], start=True, stop=True)
            nc.scalar.copy(s2s[:], s2_ps[:])
            denom = float(NT) * float(NT) - float(NT)
            a_const = float(2.0 * NT / (4.0 * denom))
            b_const = float(-2.0 / (4.0 * denom))
            nc.vector.tensor_scalar_mul(t1s[:], sqsum[:], a_const)
            nc.vector.tensor_scalar(
                out=bws[:], in0=s2s[:], scalar1=b_const, scalar2=t1s[0:1, 0:1],
                op0=ALU.mult, op1=ALU.add,
            )
            nc.vector.reciprocal(bwinv[:], bws[:])
            nc.vector.tensor_scalar_mul(vec5[:], lad_sb[:], bwinv[0:1, 0:1])
            nc.gpsimd.partition_broadcast(vec5b[:], vec5[:])
            nc.vector.tensor_scalar_mul(scales5[:], vec5b[:], 2.0)
            for p in range(KERNEL_NUM):
                nc.vector.tensor_scalar(
                    out=bias_all[:, bass.ts(p, MT)],
                    in0=bias_src[:],
                    scalar1=vec5b[:, p : p + 1],
                    scalar2=2.0,
                    op0=ALU.mult,
                    op1=ALU.mult,
                )

            # ---------- discriminator collapse + adv partials ----------
            # emitted after the collective so the gpsimd broadcasts can't
            # delay the AllGather trigger
            for ch in range(4):
                w_ps = spsum.tile([1, 512], f32, tag="sps")
                nc.tensor.matmul(
                    w_ps[:], W2T_sb[:], W1_sb[:, bass.ts(ch, 512)],
                    start=True, stop=True,
                )
                nc.scalar.copy(w_sb[:, bass.ts(ch, 512)], w_ps[:])
            beta_ps = spsum.tile([1, 1], f32, tag="sps")
            nc.tensor.matmul(beta_ps[:], W2T_sb[:], b1_sb[:], start=True, stop=True)
            nc.vector.tensor_scalar_add(beta_sb[:], beta_ps[:], b2_sb[0:1, 0:1])
            nc.gpsimd.partition_broadcast(betab[:], beta_sb[:])
            nc.gpsimd.partition_broadcast(wb[:], w_sb[:])
            for mt in range(MT):
                rt2 = rowp.tile([128, D], bf16, tag="rt")
                nc.sync.dma_start(rt2[:], rowsbf_r[mt])
                nc.vector.tensor_tensor(
                    out=junk3[:], in0=rt2[:], in1=wb[:], op=ALU.mult
                )
                nc.vector.tensor_reduce(
                    zraw[:, mt : mt + 1], junk3[:],
                    axis=mybir.AxisListType.X, op=ALU.add,
                )
            nc.vector.tensor_scalar_add(zcols[:], zraw[:], betab[:])
            nc.scalar.activation(ecols[:], zcols[:], AF.Exp, bias=0.0, scale=sgn_sb[:])
            nc.scalar.activation(
                lncols[:], ecols[:], AF.Ln, bias=1.0, scale=1.0,
                accum_out=fin[:, 2:3],
            )

            # ---------- main loop ----------
            totalT_r = totalT_d.ap().rearrange(
                "(kp two p) n -> p kp two n", p=128, two=2
            )
            DR = mybir.MatmulPerfMode.DoubleRow
            for n4 in range(NQUAD):
                rq = []
                for q in range(4):
                    rqt = rhsp.tile([128, KP, 2, NCHUNK], fp8, tag="rhs")
                    nc.sync.dma_start(
                        rqt[:], totalT_r[:, :, :, bass.ts(4 * n4 + q, NCHUNK)]
                    )
                    rq.append(rqt)
                for m in range(MT):
                    x2 = x2p.tile([128, 4 * NCHUNK], f32)
                    for half in range(2):
                        gt = gpsum.tile([128, 2 * NCHUNK], f32)
                        for kp in range(KP):
                            lw = lhsT_all[:, kp, :, bass.ts(m, 128)]
                            nc.tensor.matmul(
                                gt[:, 0:NCHUNK],
                                lw,
                                rq[2 * half][:, kp, :, :],
                                start=(kp == 0), stop=(kp == KP - 1),
                                perf_mode=DR,
                            )
                            nc.tensor.matmul(
                                gt[:, NCHUNK : 2 * NCHUNK],
                                lw,
                                rq[2 * half + 1][:, kp, :, :],
                                start=(kp == 0), stop=(kp == KP - 1),
                                perf_mode=DR,
                            )
                        nc.vector.tensor_copy(
                            x2[:, half * 1024 : (half + 1) * 1024], gt[:]
                        )
                    nc.vector.tensor_add(
                        x2[:], x2[:], colsq[:, bass.ts(n4, 4 * NCHUNK)]
                    )
                    # p=1..4 on ScalarE; p=0 = (p=1 result)^2 on VectorE
                    e1t = e1p.tile([128, 4 * NCHUNK], bf16)
                    for p in range(1, KERNEL_NUM):
                        ej = e1t if p == 1 else ejp.tile(
                            [128, 4 * NCHUNK], bf16, name="ej"
                        )
                        slot = (n4 * MT + m) * KERNEL_NUM + p
                        nc.scalar.activation(
                            ej[:],
                            x2[:],
                            AF.Exp,
                            bias=bias_all[:, p * MT + m : p * MT + m + 1],
                            scale=scales5[:, p : p + 1],
                            accum_out=accs[:, slot : slot + 1],
                        )
                    e0t = e0p.tile([128, 4 * NCHUNK], bf16)
                    nc.vector.tensor_tensor(
                        out=e0t[:], in0=e1t[:], in1=e1t[:], op=ALU.mult
                    )
                    slot0 = (n4 * MT + m) * KERNEL_NUM
                    nc.vector.tensor_reduce(
                        accs[:, slot0 : slot0 + 1], e0t[:],
                        axis=mybir.AxisListType.X, op=ALU.add,
                    )

            # ---------- epilogue: block sums -> scalars ----------
            half = (NQUAD // 2) * MT * KERNEL_NUM
            nc.vector.tensor_reduce(
                fin[:, 0:1], accs[:, :half], axis=mybir.AxisListType.X, op=ALU.add
            )
            nc.vector.tensor_reduce(
                fin[:, 1:2], accs[:, half:], axis=mybir.AxisListType.X, op=ALU.add
            )
            fin_ps = spsum.tile([4, 1], f32, tag="sps")
            nc.tensor.matmul(fin_ps[:], fin[:], ones_f[:], start=True, stop=True)
            nc.scalar.copy(outsb[:], fin_ps[:])
            nc.sync.dma_start(out_d[0, 0:4], outsb[:])
            nc.sync.dma_start(out_d[0, 4:5], bws[:])
            nc.sync.dma_start(out_d[0, 5:6], sqsum[:])
            nc.sync.dma_start(out_d[0, 6:7], s2s[:])
            nc.sync.dma_start(out_d[0, 7:8], beta_sb[:])

    nc.compile()
    return nc


def _get_module():
    if "nc" not in _cached:
        _cached["nc"] = _build_module()
    return _cached["nc"]


def kernel(source, target, W1, b1, W2, b2, _trace=False, _trace_kwargs=None):
    import concourse.bass_utils as bass_utils

    nc = _get_module()

    total = np.concatenate(
        [np.asarray(source, np.float32), np.asarray(target, np.float32)], axis=0
    )
    tot8 = total.astype(ml_dtypes.float8_e4m3)
    tot_bf = total.astype(ml_dtypes.bfloat16)
    totalT = np.ascontiguousarray(tot8.T)  # [D, NT] fp8, shared

    W1f = np.asarray(W1, np.float32)
    W2f = np.asarray(W2, np.float32)
    b1f = np.asarray(b1, np.float32).reshape(128, 1)
    b2f = np.asarray(b2, np.float32).reshape(1, 1)
    W2T = np.ascontiguousarray(W2f.reshape(1, 128).T)
    ladder = (0.5 ** np.arange(KERNEL_NUM, dtype=np.float32)).reshape(1, -1)

    in_maps = []
    for c in range(N_CORES):
        rows = np.ascontiguousarray(tot8[c * RPC : (c + 1) * RPC])
        rowsbf = np.ascontiguousarray(tot_bf[c * RPC : (c + 1) * RPC])
        blockT = np.ascontiguousarray(totalT[:, c * RPC : (c + 1) * RPC])
        sgn = np.full((128, 1), -1.0 if c < N_CORES // 2 else 1.0, np.float32)
        in_maps.append(
            {
                "totalT": totalT,
                "blockT": blockT,
                "rows": rows,
                "rowsbf": rowsbf,
                "w1": W1f,
                "w2t": W2T,
                "b1c": b1f,
                "b2c": b2f,
                "sgn": sgn,
                "ladder": ladder,
            }
        )

    kwargs = dict(_trace_kwargs or {})
    res = bass_utils.run_bass_kernel_spmd(
        nc, in_maps, core_ids=list(range(N_CORES)), trace=_trace, **kwargs
    )
    outs = [r["out"][0] for r in res.results]

    SL = [float(o[0]) for o in outs]
    SR = [float(o[1]) for o in outs]
    SA = [float(o[2]) for o in outs]

    h = N_CORES // 2
    sxx = sum(SL[:h])
    syx = sum(SL[h:])
    sxy = sum(SR[:h])
    syy = sum(SR[h:])
    loss = np.float32((sxx + syy - sxy - syx) / (float(B) * float(B)))
    adv = np.float32(sum(SA) / float(NT))

    if _trace:
        kernel._last_results = res
    return (np.asarray(loss, np.float32), np.asarray(adv, np.float32))


# revision 31
# speedup vs baseline: 1.1557x; 1.1557x over previous
"""Adversarial-MMD loss (nn_Advmmd) on 8 Trainium2 NeuronCores via Bass/Tile.

Math (eval mode, lamb=0):
  adv:  the discriminator is Linear(2048,128) -> Dropout(eval) -> Linear(128,1)
        with NO nonlinearity, so it collapses to a single linear functional
        z = x.w + beta with w = W2@W1 [2048], beta = W2@b1 + b2.
        adv_loss = 0.5*(mean log(1+exp(-z_src)) + mean log(1+exp(+z_tgt)))
  mmd:  total = [source;target] [8192,2048]; L2_ij = sq_i + sq_j - 2 G_ij with
        G = total@total.T;  bandwidth bw = sum(L2)/(n^2-n)/4 where
        sum(L2) = 2n*sum(sq) - 2*||sum_j total_j||^2 (exact identity);
        K = sum_{p=0..4} exp(-L2/(bw*2^p));
        loss = mean K[XX] + mean K[YY] - mean K[XY] - mean K[YX].

Distribution: data-parallel over Gram rows.  Core c owns 1024 rows; it
computes its [1024, 8192] Gram block in bf16 on the PE (fp32 accumulate),
applies the five Gaussian kernels on the Scalar engine (exp with
per-partition scale/bias; row sums come free via accum_out) and reduces to
two scalars (left/right half block sums).  Row norms and column sums are
exchanged with a single small AllGather; everything else is local.

Row-tile loads for the stats phase use a strided partition map
(partition p <-> local row p*8+mt) so the per-core stats land contiguously
in DRAM for the collective; all sums are order invariant.

The pipeline drains PE's PSUM through a plain copy into SBUF tiles, so the
matmul stream never waits for the collective; the -sq_j/2 column correction
is added in-place afterwards, once the AllGather lands.
"""

import numpy as np
import ml_dtypes

N_CORES = 8
B = 4096
D = 2048
NT = 2 * B            # 8192 total rows
RPC = NT // N_CORES   # 1024 rows per core
MT = RPC // 128       # 8 m-tiles per core
KT = D // 128         # 16 k-tiles
NCHUNK = 512
NQUAD = NT // (4 * NCHUNK)   # 4 groups of four 512-column chunks
KP = KT // 2                 # 8 double-row k-pairs
KERNEL_NUM = 5
CCW = RPC + D         # per-core AllGather payload: [sq (1024) | s (2048)]

_cached = {}


def _build_module():
    import concourse.bass as bass
    import concourse.tile as tile
    import concourse.mybir as mybir
    from concourse import bacc

    f32 = mybir.dt.float32
    bf16 = mybir.dt.bfloat16
    AF = mybir.ActivationFunctionType
    ALU = mybir.AluOpType

    nc = bacc.Bacc(
        "TRN2",
        target_bir_lowering=False,
        debug=False,
        enable_asserts=False,
        num_devices=N_CORES,
    )

    fp8 = mybir.dt.float8e4
    totalT_d = nc.dram_tensor("totalT", [D, NT], fp8, kind="ExternalInput")
    blockT_d = nc.dram_tensor("blockT", [D, RPC], fp8, kind="ExternalInput")
    rows_d = nc.dram_tensor("rows", [RPC, D], fp8, kind="ExternalInput")
    rowsbf_d = nc.dram_tensor("rowsbf", [RPC, D], bf16, kind="ExternalInput")
    W1_d = nc.dram_tensor("w1", [128, D], f32, kind="ExternalInput")
    W2T_d = nc.dram_tensor("w2t", [128, 1], f32, kind="ExternalInput")
    b1_d = nc.dram_tensor("b1c", [128, 1], f32, kind="ExternalInput")
    b2_d = nc.dram_tensor("b2c", [1, 1], f32, kind="ExternalInput")
    sgn_d = nc.dram_tensor("sgn", [128, 1], f32, kind="ExternalInput")
    lad_d = nc.dram_tensor("ladder", [1, KERNEL_NUM], f32, kind="ExternalInput")
    out_d = nc.dram_tensor("out", [1, 8], f32, kind="ExternalOutput")

    rg = [list(range(N_CORES))]

    with tile.TileContext(nc) as tc:
        with (
            tc.tile_pool(name="big", bufs=1) as big,
            tc.tile_pool(name="rhsp", bufs=7) as rhsp,
            tc.tile_pool(name="x2p", bufs=6) as x2p,
            tc.tile_pool(name="e1p", bufs=2) as e1p,
            tc.tile_pool(name="e0p", bufs=2) as e0p,
            tc.tile_pool(name="smalls", bufs=1) as smalls,
            tc.tile_pool(name="gpsum", bufs=3, space="PSUM") as gpsum,
            tc.tile_pool(name="spsum", bufs=1, space="PSUM") as spsum,
            tc.tile_pool(name="dram", bufs=1, space="DRAM") as dram,
            tc.tile_pool(name="prol", bufs=1) as prol,
            tc.tile_pool(name="rowp", bufs=2) as rowp,
        ):
            # ---------- persistent tiles ----------
            lhsT_all = big.tile([128, KP, 2, RPC], fp8)     # 16KB/p
            colsq = big.tile([128, NT], f32)                # 32KB/p
            accs = big.tile([128, NQUAD * MT * KERNEL_NUM], f32)
            rowsq = smalls.tile([128, MT], f32)
            bias_all = smalls.tile([128, KERNEL_NUM * MT], f32)
            scales5 = smalls.tile([128, KERNEL_NUM], f32)
            vec5b = smalls.tile([128, KERNEL_NUM], f32)
            zcols = smalls.tile([128, MT], f32)
            zraw = smalls.tile([128, MT], f32)
            ecols = smalls.tile([128, MT], f32)
            lncols = smalls.tile([128, MT], f32)
            fin = smalls.tile([128, 4], f32)
            ones_8 = smalls.tile([128, 1], fp8)
            ones_f = smalls.tile([128, 1], f32)
            sgn_sb = smalls.tile([128, 1], f32)
            lad_sb = smalls.tile([1, KERNEL_NUM], f32)
            vec5 = smalls.tile([1, KERNEL_NUM], f32)
            sqsum = smalls.tile([1, 1], f32)
            s2s = smalls.tile([1, 1], f32)
            t1s = smalls.tile([1, 1], f32)
            bws = smalls.tile([1, 1], f32)
            bwinv = smalls.tile([1, 1], f32)
            beta_sb = smalls.tile([1, 1], f32)
            betab = smalls.tile([128, 1], f32)
            outsb = smalls.tile([4, 1], f32)

            # ---------- prologue tiles ----------
            W1_sb = prol.tile([128, D], f32)                # 8KB/p
            W2T_sb = prol.tile([128, 1], f32)
            b1_sb = prol.tile([128, 1], f32)
            b2_sb = prol.tile([1, 1], f32)
            wb = prol.tile([128, D], bf16)                  # 4KB/p
            w_sb = prol.tile([1, D], bf16)
            s2d_sb = prol.tile([128, KT], f32)
            bias_src = prol.tile([128, MT], f32)
            s3v = prol.tile([128, N_CORES, KT], f32)
            s_glob = prol.tile([128, KT], f32)
            sprod = prol.tile([128, KT], f32)
            sprod_r = prol.tile([128, 1], f32)
            junk2 = prol.tile([128, D], bf16)               # 4KB/p
            junk3 = prol.tile([128, D], bf16)               # 4KB/p

            # ---------- DRAM collective buffers ----------
            cc_in = dram.tile([CCW], f32)
            cc_out = dram.tile([N_CORES * CCW], f32, addr_space="Shared")

            # ---------- constant + data loads ----------
            nc.gpsimd.dma_start(W1_sb[:], W1_d[:, :])
            nc.gpsimd.dma_start(W2T_sb[:], W2T_d[:, :])
            nc.gpsimd.dma_start(b1_sb[:], b1_d[:, :])
            nc.gpsimd.dma_start(b2_sb[:], b2_d[:, :])
            nc.gpsimd.dma_start(sgn_sb[:], sgn_d[:, :])
            nc.gpsimd.dma_start(lad_sb[:], lad_d[:, :])
            nc.vector.memset(ones_8[:], 1.0)
            nc.vector.memset(ones_f[:], 1.0)
            nc.vector.memset(fin[:, 3:4], 0.0)
            nc.sync.dma_start(
                lhsT_all[:],
                blockT_d.ap().rearrange("(kp two p) m -> p kp two m", p=128, two=2),
            )

            # ---------- local stats: sq_i (ACT Square) and s (PE) ----------
            # rows are host-permuted so local row index r = p*MT + m lives at
            # partition p, slot m -> the stats DMA out is contiguous
            s_ps = spsum.tile([128, KT], f32, tag="sps")
            rows_r = rows_d.ap().rearrange("(p m) k -> m p k", m=MT)
            rowsbf_r = rowsbf_d.ap().rearrange("(p m) k -> m p k", m=MT)
            for mt in range(MT):
                rt = rowp.tile([128, D], fp8, tag="rt8")
                nc.sync.dma_start(rt[:], rows_r[mt])
                nc.scalar.activation(
                    junk2[:], rt[:], AF.Square, bias=0.0, scale=1.0,
                    accum_out=rowsq[:, mt : mt + 1],
                )
                for kt in range(KT):
                    nc.tensor.matmul(
                        s_ps[:, kt : kt + 1],
                        rt[:, bass.ts(kt, 128)],
                        ones_8[:],
                        start=(mt == 0),
                        stop=(mt == MT - 1),
                    )
            nc.scalar.copy(s2d_sb[:], s_ps[:])

            # ---------- one AllGather: [sq_perm (1024) | s (2048)] ----------
            # both input DMAs are contiguous 32/64B lines per partition
            rowsqh = prol.tile([128, MT], f32)
            nc.vector.tensor_scalar_mul(rowsqh[:], rowsq[:], -0.5)
            nc.gpsimd.dma_start(
                cc_in[0:RPC].rearrange("(p m) -> p m", p=128), rowsqh[:]
            )
            nc.gpsimd.dma_start(
                cc_in[RPC:CCW].rearrange("(p kt) -> p kt", p=128), s2d_sb[:]
            )
            nc.gpsimd.collective_compute(
                "AllGather",
                ALU.bypass,
                replica_groups=rg,
                ins=[cc_in.opt()],
                outs=[cc_out.opt()],
            )
            cc_view = cc_out.rearrange("(c w) -> c w", c=N_CORES)
            # colsq[p, j] = sq_j  (j = c*RPC + r, r contiguous inside block)
            nc.scalar.dma_start(
                colsq[:].rearrange("p (c r) -> p c r", c=N_CORES),
                cc_view[None, :, 0:RPC].broadcast_to((128, N_CORES, RPC)),
            )
            # s parts land as [p, c, kt] (contiguous 64B lines per c)
            nc.scalar.dma_start(
                s3v[:],
                cc_view[:, RPC:CCW].rearrange("c (p kt) -> p c kt", p=128),
            )
            # per-(m-tile) bias layout of our own sq, from the local cc input:
            # bias_src[i, m] = sq(row m*128+i)
            nc.scalar.dma_start(
                bias_src[:],
                cc_in[0:RPC].rearrange("(m i) -> i m", i=128),
            )

            # sq (already scaled by -1/2) in a multi-partition layout for
            # the global sum: [128, c, m] lines of 32B
            sqt = prol.tile([128, N_CORES, MT], f32)
            nc.scalar.dma_start(
                sqt[:],
                cc_view[:, 0:RPC].rearrange("c (p m) -> p c m", p=128),
            )

            # ---------- bandwidth ----------
            nc.vector.tensor_reduce(
                s_glob[:], s3v[:].rearrange("p c kt -> p kt c"),
                axis=mybir.AxisListType.X, op=ALU.add,
            )
            sqr = prol.tile([128, 1], f32)
            nc.vector.tensor_reduce(
                sqr[:], sqt[:], axis=mybir.AxisListType.XY, op=ALU.add
            )
            t2_ps = spsum.tile([1, 1], f32, tag="sps")
            nc.tensor.matmul(t2_ps[:], sqr[:], ones_f[:], start=True, stop=True)
            nc.vector.tensor_scalar_mul(sqsum[:], t2_ps[:], -2.0)
            nc.vector.tensor_tensor(
                out=sprod[:], in0=s_glob[:], in1=s_glob[:], op=ALU.mult
            )
            nc.vector.tensor_reduce(
                sprod_r[:], sprod[:], axis=mybir.AxisListType.X, op=ALU.add
            )
            s2_ps = spsum.tile([1, 1], f32, tag="sps")
            nc.tensor.matmul(s2_ps[:], sprod_r[:], ones_f[:], start=True, stop=True)
            nc.scalar.copy(s2s[:], s2_ps[:])
            denom = float(NT) * float(NT) - float(NT)
            a_const = float(2.0 * NT / (4.0 * denom))
            b_const = float(-2.0 / (4.0 * denom))
            nc.vector.tensor_scalar_mul(t1s[:], sqsum[:], a_const)
            nc.vector.tensor_scalar(
                out=bws[:], in0=s2s[:], scalar1=b_const, scalar2=t1s[0:1, 0:1],
                op0=ALU.mult, op1=ALU.add,
            )
            nc.vector.reciprocal(bwinv[:], bws[:])
            nc.vector.tensor_scalar_mul(vec5[:], lad_sb[:], bwinv[0:1, 0:1])
            nc.gpsimd.partition_broadcast(vec5b[:], vec5[:])
            nc.vector.tensor_scalar_mul(scales5[:], vec5b[:], 2.0)
            for p in range(KERNEL_NUM):
                nc.vector.tensor_scalar(
                    out=bias_all[:, bass.ts(p, MT)],
                    in0=bias_src[:],
                    scalar1=vec5b[:, p : p + 1],
                    scalar2=2.0,
                    op0=ALU.mult,
                    op1=ALU.mult,
                )

            # ---------- discriminator collapse + adv partials ----------
            # emitted after the collective so the gpsimd broadcasts can't
            # delay the AllGather trigger
            for ch in range(4):
                w_ps = spsum.tile([1, 512], f32, tag="sps")
                nc.tensor.matmul(
                    w_ps[:], W2T_sb[:], W1_sb[:, bass.ts(ch, 512)],
                    start=True, stop=True,
                )
                nc.scalar.copy(w_sb[:, bass.ts(ch, 512)], w_ps[:])
            beta_ps = spsum.tile([1, 1], f32, tag="sps")
            nc.tensor.matmul(beta_ps[:], W2T_sb[:], b1_sb[:], start=True, stop=True)
            nc.vector.tensor_scalar_add(beta_sb[:], beta_ps[:], b2_sb[0:1, 0:1])
            nc.gpsimd.partition_broadcast(betab[:], beta_sb[:])
            nc.gpsimd.partition_broadcast(wb[:], w_sb[:])
            for mt in range(MT):
                rt2 = rowp.tile([128, D], bf16, tag="rt")
                nc.sync.dma_start(rt2[:], rowsbf_r[mt])
                nc.vector.tensor_tensor(
                    out=junk3[:], in0=rt2[:], in1=wb[:], op=ALU.mult
                )
                nc.vector.tensor_reduce(
                    zraw[:, mt : mt + 1], junk3[:],
                    axis=mybir.AxisListType.X, op=ALU.add,
                )
            nc.vector.tensor_scalar_add(zcols[:], zraw[:], betab[:])
            nc.scalar.activation(ecols[:], zcols[:], AF.Exp, bias=0.0, scale=sgn_sb[:])
            nc.scalar.activation(
                lncols[:], ecols[:], AF.Ln, bias=1.0, scale=1.0,
                accum_out=fin[:, 2:3],
            )

            # ---------- main loop ----------
            totalT_r = totalT_d.ap().rearrange(
                "(kp two p) n -> p kp two n", p=128, two=2
            )
            DR = mybir.MatmulPerfMode.DoubleRow
            for n4 in range(NQUAD):
                rq = []
                for q in range(4):
                    rqt = rhsp.tile([128, KP, 2, NCHUNK], fp8, tag="rhs")
                    nc.sync.dma_start(
                        rqt[:], totalT_r[:, :, :, bass.ts(4 * n4 + q, NCHUNK)]
                    )
                    rq.append(rqt)
                for m in range(MT):
                    x2 = x2p.tile([128, 4 * NCHUNK], f32)
                    for half in range(2):
                        gt = gpsum.tile([128, 2 * NCHUNK], f32)
                        for kp in range(KP):
                            lw = lhsT_all[:, kp, :, bass.ts(m, 128)]
                            nc.tensor.matmul(
                                gt[:, 0:NCHUNK],
                                lw,
                                rq[2 * half][:, kp, :, :],
                                start=(kp == 0), stop=(kp == KP - 1),
                                perf_mode=DR,
                            )
                            nc.tensor.matmul(
                                gt[:, NCHUNK : 2 * NCHUNK],
                                lw,
                                rq[2 * half + 1][:, kp, :, :],
                                start=(kp == 0), stop=(kp == KP - 1),
                                perf_mode=DR,
                            )
                        nc.vector.tensor_copy(
                            x2[:, half * 1024 : (half + 1) * 1024], gt[:]
                        )
                    nc.vector.tensor_add(
                        x2[:], x2[:], colsq[:, bass.ts(n4, 4 * NCHUNK)]
                    )
                    # p=2..4 on ScalarE; p=1 = e2^2 and p=0 = e1^2 on
                    # VectorE via fused square+row-sum (scalar_tensor_tensor)
                    e2t = e1p.tile([128, 4 * NCHUNK], bf16, name="e2t")
                    for p in range(2, KERNEL_NUM):
                        ej = e2t if p == 2 else e0p.tile(
                            [128, 4 * NCHUNK], bf16, name="e0t", tag="e0t"
                        )
                        slot = (n4 * MT + m) * KERNEL_NUM + p
                        nc.scalar.activation(
                            ej[:],
                            x2[:],
                            AF.Exp,
                            bias=bias_all[:, p * MT + m : p * MT + m + 1],
                            scale=scales5[:, p : p + 1],
                            accum_out=accs[:, slot : slot + 1],
                        )
                    base = (n4 * MT + m) * KERNEL_NUM
                    e1t = e0p.tile([128, 4 * NCHUNK], bf16, name="e1t", tag="e0t")
                    nc.vector.scalar_tensor_tensor(
                        out=e1t[:], in0=e2t[:], scalar=1.0, in1=e2t[:],
                        op0=ALU.mult, op1=ALU.mult,
                        accum_out=accs[:, base + 1 : base + 2],
                    )
                    e0t = e0p.tile([128, 4 * NCHUNK], bf16, name="e0t", tag="e0t")
                    nc.vector.scalar_tensor_tensor(
                        out=e0t[:], in0=e1t[:], scalar=1.0, in1=e1t[:],
                        op0=ALU.mult, op1=ALU.mult,
                        accum_out=accs[:, base : base + 1],
                    )

            # ---------- epilogue: block sums -> scalars ----------
            half = (NQUAD // 2) * MT * KERNEL_NUM
            nc.vector.tensor_reduce(
                fin[:, 0:1], accs[:, :half], axis=mybir.AxisListType.X, op=ALU.add
            )
            nc.vector.tensor_reduce(
                fin[:, 1:2], accs[:, half:], axis=mybir.AxisListType.X, op=ALU.add
            )
            fin_ps = spsum.tile([4, 1], f32, tag="sps")
            nc.tensor.matmul(fin_ps[:], fin[:], ones_f[:], start=True, stop=True)
            nc.scalar.copy(outsb[:], fin_ps[:])
            nc.sync.dma_start(out_d[0, 0:4], outsb[:])
            nc.sync.dma_start(out_d[0, 4:5], bws[:])
            nc.sync.dma_start(out_d[0, 5:6], sqsum[:])
            nc.sync.dma_start(out_d[0, 6:7], s2s[:])
            nc.sync.dma_start(out_d[0, 7:8], beta_sb[:])

    nc.compile()
    return nc


def _get_module():
    if "nc" not in _cached:
        _cached["nc"] = _build_module()
    return _cached["nc"]


def kernel(source, target, W1, b1, W2, b2, _trace=False, _trace_kwargs=None):
    import concourse.bass_utils as bass_utils

    nc = _get_module()

    total = np.concatenate(
        [np.asarray(source, np.float32), np.asarray(target, np.float32)], axis=0
    )
    tot8 = total.astype(ml_dtypes.float8_e4m3)
    tot_bf = total.astype(ml_dtypes.bfloat16)
    totalT = np.ascontiguousarray(tot8.T)  # [D, NT] fp8, shared

    W1f = np.asarray(W1, np.float32)
    W2f = np.asarray(W2, np.float32)
    b1f = np.asarray(b1, np.float32).reshape(128, 1)
    b2f = np.asarray(b2, np.float32).reshape(1, 1)
    W2T = np.ascontiguousarray(W2f.reshape(1, 128).T)
    ladder = (0.5 ** np.arange(KERNEL_NUM, dtype=np.float32)).reshape(1, -1)

    in_maps = []
    for c in range(N_CORES):
        rows = np.ascontiguousarray(tot8[c * RPC : (c + 1) * RPC])
        rowsbf = np.ascontiguousarray(tot_bf[c * RPC : (c + 1) * RPC])
        blockT = np.ascontiguousarray(totalT[:, c * RPC : (c + 1) * RPC])
        sgn = np.full((128, 1), -1.0 if c < N_CORES // 2 else 1.0, np.float32)
        in_maps.append(
            {
                "totalT": totalT,
                "blockT": blockT,
                "rows": rows,
                "rowsbf": rowsbf,
                "w1": W1f,
                "w2t": W2T,
                "b1c": b1f,
                "b2c": b2f,
                "sgn": sgn,
                "ladder": ladder,
            }
        )

    kwargs = dict(_trace_kwargs or {})
    res = bass_utils.run_bass_kernel_spmd(
        nc, in_maps, core_ids=list(range(N_CORES)), trace=_trace, **kwargs
    )
    outs = [r["out"][0] for r in res.results]

    SL = [float(o[0]) for o in outs]
    SR = [float(o[1]) for o in outs]
    SA = [float(o[2]) for o in outs]

    h = N_CORES // 2
    sxx = sum(SL[:h])
    syx = sum(SL[h:])
    sxy = sum(SR[:h])
    syy = sum(SR[h:])
    loss = np.float32((sxx + syy - sxy - syx) / (float(B) * float(B)))
    adv = np.float32(sum(SA) / float(NT))

    if _trace:
        kernel._last_results = res
    return (np.asarray(loss, np.float32), np.asarray(adv, np.float32))
